# revision 8
# baseline (speedup 1.0000x reference)
"""Criss-cross attention block (CCNet) Bass/Tile kernel for Trainium2.

Shapes (hardcoded): B=8, C=256, H=W=128, CR=32. Data-parallel over batch:
core b processes image b. Full inputs in, full output out.
"""
import sys

sys.path.insert(0, "/opt/trn_rl_repo")

import numpy as np
import ml_dtypes

import concourse.bass as bass
import concourse.mybir as mybir
from concourse import bacc, tile
from concourse.bass_utils import run_bass_kernel_spmd

B, C, H, W, CR = 8, 256, 128, 128, 32
HW = H * W
BF = ml_dtypes.bfloat16

_BUILD_CACHE = {}


def _build(with_qkv_bias: bool, with_z_bias: bool):
    """Build + compile the per-core program (SPMD, same NEFF on all 8 cores)."""
    nc = bacc.Bacc("TRN2", target_bir_lowering=False, debug=False, num_devices=8)
    dt = mybir.dt
    f32, bf16 = dt.float32, dt.bfloat16

    x_d = nc.dram_tensor("x", [C, HW], f32, kind="ExternalInput").ap()
    wqkvT_d = nc.dram_tensor("wqkvT", [C, 96], bf16, kind="ExternalInput").ap()
    wzT_d = nc.dram_tensor("wzT", [CR, C], bf16, kind="ExternalInput").ap()
    mask_d = nc.dram_tensor("mask4", [128, 4, 128], bf16, kind="ExternalInput").ap()
    ident_d = nc.dram_tensor("identpad", [128, 32], bf16, kind="ExternalInput").ap()
    if with_qkv_bias:
        bqkv_d = nc.dram_tensor("bqkv", [1, 96], bf16, kind="ExternalInput").ap()
    if with_z_bias:
        bzr_d = nc.dram_tensor("bz_row", [1, C], bf16, kind="ExternalInput").ap()

    zscr = nc.dram_tensor("zscr", [HW], bf16, kind="Internal").ap()
    rscr = nc.dram_tensor("rscr", [HW], f32, kind="Internal").ap()
    out_d = nc.dram_tensor("out", [C, HW], f32, kind="ExternalOutput").ap()

    with tile.TileContext(nc) as tc:
        with (
            tc.tile_pool(name="persist", bufs=1) as pp,
            tc.tile_pool(name="work", bufs=3) as wp,
            tc.tile_pool(name="outw", bufs=3) as op,
        ):
            # ---- persistent SBUF tensors ----
            x_bf = pp.tile([128, 2, HW], bf16)       # x cast to bf16, 2 c-halves
            qkv = pp.tile([96, H, W], bf16)          # Q rows 0-31, K 32-63, V 64-95
            k2 = pp.tile([32, H, W], bf16)           # K shifted to base partition 0
            out_u = pp.tile([33, H, W], bf16)        # rows 0-31 attn out, row 32 = Z
            vt = pp.tile([128, W, 33], bf16)         # V^T col stripes + ones col
            vt2 = pp.tile([128, H, 33], bf16)        # V^T row stripes + ones col
            wqkvT = pp.tile([128, 2, 96], bf16)
            wzT = pp.tile([CR, C], bf16)
            mask4 = pp.tile([128, 4, 128], bf16)
            ident = pp.tile([128, 32], bf16)

            # ---- const loads ----
            nc.sync.dma_start(out=wqkvT[:], in_=wqkvT_d.rearrange("(a p) m -> p a m", p=128))
            nc.sync.dma_start(out=wzT[:], in_=wzT_d)
            nc.sync.dma_start(out=mask4[:], in_=mask_d)
            nc.sync.dma_start(out=ident[:], in_=ident_d)
            if with_qkv_bias or with_z_bias:
                ones_row = pp.tile([1, 512], bf16)
                nc.vector.memset(ones_row[:], 1.0)
            if with_qkv_bias:
                bqkv = pp.tile([1, 96], bf16)
                nc.sync.dma_start(out=bqkv[:], in_=bqkv_d)
            if with_z_bias:
                bz_row = pp.tile([1, C], bf16)
                nc.sync.dma_start(out=bz_row[:], in_=bzr_d)

            nc.vector.memset(vt[:, :, 32:33], 1.0)
            nc.vector.memset(vt2[:, :, 32:33], 1.0)

            # ---- load x (cast f32 -> bf16 in DMA, SWDGE), 4 quarters x 2 halves ----
            for q in range(4):
                s = q * 4096
                for half in range(2):
                    nc.gpsimd.dma_start(
                        out=x_bf[:, half, s:s + 4096],
                        in_=x_d[half * 128:(half + 1) * 128, s:s + 4096],
                    )

            # ================= P1: QKV projections =================
            with tc.tile_pool(name="psbig", bufs=3, space="PSUM") as pb:
                for ch in range(32):
                    s = ch * 512
                    ps = pb.tile([96, 512], f32, tag="ps1")
                    nc.tensor.matmul(ps[:], wqkvT[:, 0, :], x_bf[:, 0, s:s + 512],
                                     start=True, stop=False)
                    nc.tensor.matmul(ps[:], wqkvT[:, 1, :], x_bf[:, 1, s:s + 512],
                                     start=False, stop=not with_qkv_bias)
                    if with_qkv_bias:
                        nc.tensor.matmul(ps[:], bqkv[:], ones_row[:],
                                         start=False, stop=True)
                    h0 = ch * 4
                    # Q,K,V -> qkv (lane-aligned, ACT); K -> k2 (partition shift, DVE)
                    nc.scalar.copy(qkv[:, h0:h0 + 4, :], ps[:].rearrange("p (a b) -> p a b", b=128))
                    nc.vector.tensor_copy(k2[:, h0:h0 + 4, :],
                                          ps[32:64, :].rearrange("p (a b) -> p a b", b=128))

            with tc.tile_pool(name="ps2k", bufs=6, space="PSUM") as ps2:
                # ================= P1b: V^T column stripes (for col pass) ======
                for wb in range(32):
                    w0 = wb * 4
                    pv = ps2.tile([128, 4, 32], bf16, tag="ps")
                    for j in range(4):
                        nc.tensor.transpose(pv[:, j, :], qkv[64:96, :, w0 + j],
                                            ident[64:96, :])
                    nc.vector.tensor_copy(vt[:, w0:w0 + 4, 0:32], pv[:])

                # ================= P2: column attention (masked diag) ==========
                prev = None  # (ps_a, wb) pipeline by one batch
                for wb in range(32):
                    w0 = wb * 4
                    ps_e = ps2.tile([128, 4, 128], f32, tag="ps")
                    for j in range(4):
                        nc.tensor.matmul(ps_e[:, j, :], k2[:, :, w0 + j],
                                         qkv[0:32, :, w0 + j], start=True, stop=True)
                    expe = wp.tile([128, 4, 128], bf16, tag="expe")
                    nc.scalar.activation(expe[:], ps_e[:], mybir.ActivationFunctionType.Exp)
                    nc.vector.tensor_mul(expe[:], expe[:], mask4[:])
                    ps_a = ps2.tile([33, 4, 128], f32, tag="ps")
                    for j in range(4):
                        nc.tensor.matmul(ps_a[:, j, :], vt[:, w0 + j, :], expe[:, j, :],
                                         start=True, stop=True)
                    # write out_u[:, :, w0:w0+4] in (w, h) iteration order
                    dst = out_u[:, :, w0:w0 + 4].rearrange("p h w -> p w h")
                    nc.vector.tensor_copy(dst, ps_a[:])

                # ================= P1c: V^T row stripes (for row pass) =========
                for hb in range(32):
                    h0 = hb * 4
                    pv = ps2.tile([128, 4, 32], bf16, tag="ps")
                    for j in range(4):
                        nc.tensor.transpose(pv[:, j, :], qkv[64:96, h0 + j, :],
                                            ident[64:96, :])
                    nc.vector.tensor_copy(vt2[:, h0:h0 + 4, 0:32], pv[:])

                # ================= P3: row attention (no mask) =================
                for hb in range(32):
                    h0 = hb * 4
                    ps_e = ps2.tile([128, 4, 128], f32, tag="ps")
                    for j in range(4):
                        nc.tensor.matmul(ps_e[:, j, :], k2[:, h0 + j, :],
                                         qkv[0:32, h0 + j, :], start=True, stop=True)
                    expe = wp.tile([128, 4, 128], bf16, tag="expe")
                    nc.scalar.activation(expe[:], ps_e[:], mybir.ActivationFunctionType.Exp)
                    ps_a = ps2.tile([33, 4, 128], f32, tag="ps")
                    for j in range(4):
                        nc.tensor.matmul(ps_a[:, j, :], vt2[:, h0 + j, :], expe[:, j, :],
                                         start=True, stop=True)
                    dst = out_u[:, h0:h0 + 4, :]
                    nc.vector.tensor_add(dst, dst, ps_a[:])

                # ================= P4: Z -> 1/Z =================
                nc.sync.dma_start(out=zscr.rearrange("(p f) -> p f", p=1),
                                  in_=out_u[32:33, :, :].rearrange("p a b -> p (a b)"))
                zsq = wp.tile([128, 128], bf16, tag="zsq")
                nc.sync.dma_start(out=zsq[:], in_=zscr.rearrange("(p f) -> p f", p=128))
                rsq = wp.tile([128, 128], f32, tag="rsq")
                nc.vector.reciprocal(rsq[:], zsq[:])
                nc.sync.dma_start(out=rscr.rearrange("(p f) -> p f", p=128), in_=rsq[:])

                # ================= P5: Wz projection + bias + residual =========
                for cch in range(32):
                    px = cch * 512
                    h0 = cch * 4
                    rb = wp.tile([32, 4, 128], f32, tag="rb")
                    src = rscr[px:px + 512].rearrange("(a b) -> a b", b=128)
                    bcast = bass.AP(tensor=src.tensor, offset=src.offset,
                                    ap=[[0, 32]] + list(src.ap))
                    nc.sync.dma_start(out=rb[:], in_=bcast)
                    norm = wp.tile([32, 4, 128], bf16, tag="norm")
                    nc.vector.tensor_mul(norm[:], out_u[0:32, h0:h0 + 4, :], rb[:])
                    rhs = norm[:].rearrange("p a b -> p (a b)")
                    for half in range(2):
                        ps_f = ps2.tile([128, 512], f32, tag="ps")
                        nc.tensor.matmul(ps_f[:], wzT[:, half * 128:(half + 1) * 128],
                                         rhs, start=True, stop=not with_z_bias)
                        if with_z_bias:
                            nc.tensor.matmul(
                                ps_f[:], bz_row[:, half * 128:(half + 1) * 128],
                                ones_row[:], start=False, stop=True)
                        of = op.tile([128, 512], f32, tag="of")
                        if half == 0:
                            nc.vector.tensor_add(of[:], ps_f[:], x_bf[:, 0, px:px + 512])
                        else:
                            tmp = op.tile([128, 512], f32, tag="tmp")
                            nc.scalar.copy(tmp[:], ps_f[:])
                            nc.gpsimd.tensor_add(of[:], tmp[:], x_bf[:, 1, px:px + 512])
                        nc.sync.dma_start(out=out_d[half * 128:(half + 1) * 128, px:px + 512],
                                          in_=of[:])
    nc.compile()
    return nc


def _host_prep(Wq, bq, Wk, bk, Wv, bv, Wz, bz):
    wqkvT = np.ascontiguousarray(
        np.concatenate([Wq, Wk, Wv], axis=0).T).astype(BF)          # (256, 96)
    wzT = np.ascontiguousarray(Wz.T).astype(BF)                      # (32, 256)
    bz_row = np.asarray(bz, np.float32).reshape(1, C).astype(BF)
    eye = np.eye(128, dtype=np.float32)
    mask4 = np.ascontiguousarray(
        np.broadcast_to((1.0 - eye)[:, None, :], (128, 4, 128))).astype(BF)
    identpad = np.vstack([np.eye(32, dtype=np.float32)] * 4).astype(BF)
    bqkv = np.concatenate([bq, bk, bv]).reshape(1, 96).astype(BF)
    return wqkvT, wzT, bz_row, mask4, identpad, bqkv


def kernel(x, Wq, bq, Wk, bk, Wv, bv, Wz, bz):
    x = np.asarray(x, np.float32)
    wqkvT, wzT, bz_row, mask4, identpad, bqkv = _host_prep(
        np.asarray(Wq, np.float32), np.asarray(bq, np.float32),
        np.asarray(Wk, np.float32), np.asarray(bk, np.float32),
        np.asarray(Wv, np.float32), np.asarray(bv, np.float32),
        np.asarray(Wz, np.float32), np.asarray(bz, np.float32))
    with_qkv_bias = bool(np.any(bqkv.astype(np.float32) != 0.0))
    with_z_bias = bool(np.any(bz_row.astype(np.float32) != 0.0))

    key = (with_qkv_bias, with_z_bias)
    if key not in _BUILD_CACHE:
        _BUILD_CACHE[key] = _build(*key)
    nc = _BUILD_CACHE[key]

    in_maps = []
    for b in range(B):
        m = dict(
            x=np.ascontiguousarray(x[b].reshape(C, HW)),
            wqkvT=wqkvT, wzT=wzT, mask4=mask4, identpad=identpad,
        )
        if with_qkv_bias:
            m["bqkv"] = bqkv
        if with_z_bias:
            m["bz_row"] = bz_row
        in_maps.append(m)

    res = run_bass_kernel_spmd(nc, in_maps, core_ids=list(range(8)))
    out = np.stack([res.results[b]["out"].reshape(C, H, W) for b in range(B)])
    return out
